# revision 7
# baseline (speedup 1.0000x reference)
"""Block-diagonal 1x1x1 conv (LocalityAdaptive) Trainium2 Bass kernel.

x: [4, 192, 192, 192, 3] f32, kernel: [6, 6, 6, 3, 1] f32 (per-32^3-block
channel-mixing weights), out: [4, 192, 192, 192, 1] f32.

Sharding: 8 cores = (batch n, D-half). Each core gets the contiguous slab
x[n, d0:d0+96] viewed as [18432 rows, 576] (row = (d', h), col = (w, c)),
plus host-precomputed per-partition weight tiles. On-core: DMA 128-row
tiles in groups of 12, multiply by the (partition, col)-aligned weight
tile, then sum each 3-channel group along the free dim with two strided
adds. All DVE work overlaps the HBM-bound DMA stream.
"""

import numpy as np

ROI = 32
N, D, H, W, C = 4, 192, 192, 192, 3
DSH = D // 2               # 96 d-rows per core
ROWS = DSH * H             # 18432
WC = W * C                 # 576
NT = ROWS // 128           # 144 tiles of 128 rows
G = 12                     # tiles per group (must be multiple of 3, divide 48)
NG = NT // G               # 12 groups
TRIP = 3 * WC              # 1728 cols per tile-triplet

_prog = None


def _build_program():
    import concourse.tile as tile
    from concourse import bacc, mybir

    nc = bacc.Bacc("TRN2", target_bir_lowering=False, debug=False)
    f32 = mybir.dt.float32
    x = nc.dram_tensor("x", [ROWS, WC], f32, kind="ExternalInput").ap()
    w = nc.dram_tensor("w", [3, 128, TRIP], f32, kind="ExternalInput").ap()
    out = nc.dram_tensor("out", [ROWS, W], f32, kind="ExternalOutput").ap()

    xg_all = x.rearrange("(g j p) w -> g p j w", g=NG, j=G, p=128)
    og_all = out.rearrange("(g j p) w -> g p j w", g=NG, j=G, p=128)
    w_src = w.rearrange("b p k -> p b k")

    with tile.TileContext(nc) as tc:
        with tc.tile_pool(name="wpool", bufs=1) as wpool, \
             tc.tile_pool(name="xpool", bufs=4) as xpool, \
             tc.tile_pool(name="opool", bufs=3) as opool:
            w_sb = wpool.tile([128, 3, TRIP], f32)
            # Per-dblk weight loads so the first mul only waits ~2.5us.
            for b in range(3):
                # dblk 0 heads the SP queue (first mul needs it); dblk 1/2 go
                # on the ACT queue, which only carries outputs and is idle early.
                dmae = nc.sync if b == 0 else nc.scalar
                dmae.dma_start(out=w_sb[:, b, :], in_=w_src[:, b, :])
                if b == 0:
                    # DVE touches the dblk-0 slice so the first tensor_mul
                    # doesn't need a second sync-wait slot.
                    wtouch = wpool.tile([128, 1], f32)
                    nc.vector.tensor_copy(out=wtouch[:], in_=w_sb[:, 0, 0:1])
            for g in range(NG):
                b = g // (NG // 3)  # d-block index of this group
                xg = xpool.tile([128, G * WC], f32)
                xin = xg[:].rearrange("p (j w) -> p j w", j=G)
                ot = opool.tile([128, G * W], f32)
                # Group 0: chunk by triplet for fast pipeline fill.
                nchunk = G // 3 if g == 0 else 1
                for ck in range(nchunk):
                    j0, j1 = ck * G // nchunk, (ck + 1) * G // nchunk
                    nc.sync.dma_start(out=xin[:, j0:j1], in_=xg_all[g][:, j0:j1])
                    xv = xg[:, j0 * WC:j1 * WC].rearrange(
                        "p (r k) -> p r k", r=(j1 - j0) // 3)
                    nc.vector.tensor_mul(
                        out=xv, in0=xv,
                        in1=w_sb[:, b, :].unsqueeze(1).broadcast_to(
                            [128, (j1 - j0) // 3, TRIP]),
                    )
                    t3 = xg[:, j0 * WC:j1 * WC].rearrange("p (m c) -> p m c", c=C)
                    oc = ot[:, j0 * W:j1 * W]
                    nc.vector.tensor_add(out=oc, in0=t3[:, :, 0], in1=t3[:, :, 1])
                    nc.gpsimd.tensor_add(out=oc, in0=oc, in1=t3[:, :, 2])
                nc.scalar.dma_start(
                    out=og_all[g],
                    in_=ot[:].rearrange("p (j w) -> p j w", j=G),
                )
    nc.compile()
    return nc


def _weight_tiles(kern, gdb0):
    """Per-partition weight tiles for one core: [3 dblk, 128, 3*576].

    Tile t (128 rows) in the row stream has rows r = t*128 + p with
    h = ((t % 3)*128 + p) % 192 and d-block t//48. Column j*576 + w*3 + c of
    a triplet needs kernel[dblk, h//32, w//32, c] with j = t % 3.
    """
    wt = np.empty((3, 128, TRIP), np.float32)
    p = np.arange(128)
    for b in range(3):
        kv = kern[gdb0 + b, :, :, :, 0]                      # [6 hblk, 6 wblk, 3]
        kcols = np.repeat(kv, ROI, axis=1).reshape(6, WC)    # [6 hblk, 576]
        for j in range(3):
            hblk = ((j * 128 + p) % 192) // ROI              # [128]
            wt[b, :, j * WC:(j + 1) * WC] = kcols[hblk]
    return wt


def kernel(x, kernel):
    global _prog
    from concourse.bass_utils import run_bass_kernel_spmd

    x = np.ascontiguousarray(x, dtype=np.float32)
    kern = np.ascontiguousarray(kernel, dtype=np.float32)

    if _prog is None:
        _prog = _build_program()

    in_maps = []
    for core in range(8):
        n, half = core // 2, core % 2
        shard = x[n, half * DSH:(half + 1) * DSH].reshape(ROWS, WC)
        in_maps.append({"x": shard, "w": _weight_tiles(kern, half * 3)})

    res = run_bass_kernel_spmd(_prog, in_maps, list(range(8)))

    out = np.empty((N, D, H, W, 1), np.float32)
    for core in range(8):
        n, half = core // 2, core % 2
        out[n, half * DSH:(half + 1) * DSH, :, :, 0] = (
            res.results[core]["out"].reshape(DSH, H, W)
        )
    return out


# revision 8
# speedup vs baseline: 1.0484x; 1.0484x over previous
"""Block-diagonal 1x1x1 conv (LocalityAdaptive) Trainium2 Bass kernel.

x: [4, 192, 192, 192, 3] f32, kernel: [6, 6, 6, 3, 1] f32 (per-32^3-block
channel-mixing weights), out: [4, 192, 192, 192, 1] f32.

Sharding: 8 cores = (batch n, D-half). Each core gets the contiguous slab
x[n, d0:d0+96] viewed as [18432 rows, 576] (row = (d', h), col = (w, c)),
plus host-precomputed per-partition weight tiles. On-core: DMA 128-row
tiles in groups of 12, multiply by the (partition, col)-aligned weight
tile, then sum each 3-channel group along the free dim with two strided
adds. All DVE work overlaps the HBM-bound DMA stream.
"""

import numpy as np

ROI = 32
N, D, H, W, C = 4, 192, 192, 192, 3
DSH = D // 2               # 96 d-rows per core
ROWS = DSH * H             # 18432
WC = W * C                 # 576
NT = ROWS // 128           # 144 tiles of 128 rows
G = 12                     # tiles per group (must be multiple of 3, divide 48)
NG = NT // G               # 12 groups
TRIP = 3 * WC              # 1728 cols per tile-triplet

_prog = None


def _build_program():
    import concourse.tile as tile
    from concourse import bacc, mybir

    nc = bacc.Bacc("TRN2", target_bir_lowering=False, debug=False)
    f32 = mybir.dt.float32
    x = nc.dram_tensor("x", [ROWS, WC], f32, kind="ExternalInput").ap()
    w = nc.dram_tensor("w", [3, 128, TRIP], f32, kind="ExternalInput").ap()
    out = nc.dram_tensor("out", [ROWS, W], f32, kind="ExternalOutput").ap()

    xg_all = x.rearrange("(g j p) w -> g p j w", g=NG, j=G, p=128)
    og_all = out.rearrange("(g j p) w -> g p j w", g=NG, j=G, p=128)
    w_src = w.rearrange("b p k -> p b k")

    with tile.TileContext(nc) as tc:
        with tc.tile_pool(name="wpool", bufs=1) as wpool, \
             tc.tile_pool(name="xpool", bufs=4) as xpool, \
             tc.tile_pool(name="opool", bufs=3) as opool:
            w_sb = wpool.tile([128, 3, TRIP], f32)
            # Per-dblk weight loads so the first mul only waits ~2.5us.
            for b in range(3):
                # dblk 0 heads the SP queue (first mul needs it); dblk 1/2 go
                # on the ACT queue, which only carries outputs and is idle early.
                dmae = nc.sync if b == 0 else nc.scalar
                dmae.dma_start(out=w_sb[:, b, :], in_=w_src[:, b, :])
                if b == 0:
                    # DVE touches the dblk-0 slice so the first tensor_mul
                    # doesn't need a second sync-wait slot.
                    wtouch = wpool.tile([128, 1], f32)
                    nc.vector.tensor_copy(out=wtouch[:], in_=w_sb[:, 0, 0:1])
            for g in range(NG):
                b = g // (NG // 3)  # d-block index of this group
                xg = xpool.tile([128, G * WC], f32)
                xin = xg[:].rearrange("p (j w) -> p j w", j=G)
                ot = opool.tile([128, G * W], f32)
                # First group: chunk by triplet for fast pipeline fill.
                # Last group: chunk + per-chunk stores for a fast tail drain.
                nchunk = G // 3 if g in (0, NG - 1) else 1
                oview = ot[:].rearrange("p (j w) -> p j w", j=G)
                for ck in range(nchunk):
                    j0, j1 = ck * G // nchunk, (ck + 1) * G // nchunk
                    nc.sync.dma_start(out=xin[:, j0:j1], in_=xg_all[g][:, j0:j1])
                    xv = xg[:, j0 * WC:j1 * WC].rearrange(
                        "p (r k) -> p r k", r=(j1 - j0) // 3)
                    nc.vector.tensor_mul(
                        out=xv, in0=xv,
                        in1=w_sb[:, b, :].unsqueeze(1).broadcast_to(
                            [128, (j1 - j0) // 3, TRIP]),
                    )
                    t3 = xg[:, j0 * WC:j1 * WC].rearrange("p (m c) -> p m c", c=C)
                    oc = ot[:, j0 * W:j1 * W]
                    nc.vector.tensor_add(out=oc, in0=t3[:, :, 0], in1=t3[:, :, 1])
                    nc.gpsimd.tensor_add(out=oc, in0=oc, in1=t3[:, :, 2])
                    if nchunk > 1:
                        nc.scalar.dma_start(
                            out=og_all[g][:, j0:j1], in_=oview[:, j0:j1])
                if nchunk == 1:
                    nc.scalar.dma_start(out=og_all[g], in_=oview)
    nc.compile()
    return nc


def _weight_tiles(kern, gdb0):
    """Per-partition weight tiles for one core: [3 dblk, 128, 3*576].

    Tile t (128 rows) in the row stream has rows r = t*128 + p with
    h = ((t % 3)*128 + p) % 192 and d-block t//48. Column j*576 + w*3 + c of
    a triplet needs kernel[dblk, h//32, w//32, c] with j = t % 3.
    """
    wt = np.empty((3, 128, TRIP), np.float32)
    p = np.arange(128)
    for b in range(3):
        kv = kern[gdb0 + b, :, :, :, 0]                      # [6 hblk, 6 wblk, 3]
        kcols = np.repeat(kv, ROI, axis=1).reshape(6, WC)    # [6 hblk, 576]
        for j in range(3):
            hblk = ((j * 128 + p) % 192) // ROI              # [128]
            wt[b, :, j * WC:(j + 1) * WC] = kcols[hblk]
    return wt


def kernel(x, kernel):
    global _prog
    from concourse.bass_utils import run_bass_kernel_spmd

    x = np.ascontiguousarray(x, dtype=np.float32)
    kern = np.ascontiguousarray(kernel, dtype=np.float32)

    if _prog is None:
        _prog = _build_program()

    in_maps = []
    for core in range(8):
        n, half = core // 2, core % 2
        shard = x[n, half * DSH:(half + 1) * DSH].reshape(ROWS, WC)
        in_maps.append({"x": shard, "w": _weight_tiles(kern, half * 3)})

    res = run_bass_kernel_spmd(_prog, in_maps, list(range(8)))

    out = np.empty((N, D, H, W, 1), np.float32)
    for core in range(8):
        n, half = core // 2, core % 2
        out[n, half * DSH:(half + 1) * DSH, :, :, 0] = (
            res.results[core]["out"].reshape(DSH, H, W)
        )
    return out
